# revision 24
# baseline (speedup 1.0000x reference)
"""FAVOR+ (Performer) causal linear attention with rotary embeddings on 8 TRN2 cores.

Reference computation (B=2, L=4096, H=8, D=64, M=256):
  q,k <- GPT-J rotary(q, k, sinu_pos)
  qp = relu(rot_q @ projT / sqrt(M)) + EPS   [B,L,H,M]
  kp = relu(rot_k @ projT / sqrt(M)) + EPS
  causal scan over L: KV_l = sum_{j<=l} kp_j (x) [v_j, 1];  out_l = (qp_l @ KV_l)[:D] / (qp_l @ KV_l)[D]

Sharding: 16 (b,h) pairs, 2 per core (pure data parallel, no collectives).
Per core: chunked scan with C=128 chunks. The two pairs are interleaved
chunk-by-chunk (two independent dependency chains fill each other's
cross-engine stalls) and the feature frontend A(ci) is emitted one chunk
ahead of the state backend B(ci-1) (software pipeline).

The KV state [M, D+1] lives in PSUM (one bank per pair, both m-halves
packed at 16B-aligned offsets) and accumulates across chunks via matmul
accumulation (has_written bits; start=True only on the very first update).

Numerics: all matmul operands are bf16, accumulation fp32 in PSUM; the
final num/den division is fp32. Measured end-to-end relative error vs the
fp32 reference ~2.3e-3 (dominated by bf16 rounding of matmul operands).

Hardware notes baked in:
 - fp32 matmuls on TRN2 are emulated as 2 bf16 passes (2x instructions,
   2x weight loads) -> use bf16 operands.
 - Matmuls on disjoint PE row groups execute CONCURRENTLY; two such
   matmuls draining into the same PSUM bank crash the device. q-side
   (rows 0:63) and k-side (rows 64:127) matmuls write separate banks.
 - DMA loads and stores are issued from different HWDGE queues (SP vs ACT)
   to avoid head-of-line blocking of prefetch behind result stores.
 - This walrus build supports ONE sync-wait slot per instruction;
   _legalize_sync_waits splits multi-wait instructions.
"""

import sys
import os

for _p in ("/opt/trn_rl_repo", "/root/.axon_site/_ro/trn_rl_repo"):
    if os.path.isdir(_p) and _p not in sys.path:
        sys.path.insert(0, _p)

import numpy as np
import ml_dtypes
import concourse.bass as bass
import concourse.mybir as mybir
import concourse.tile as tile
from concourse.bass_utils import run_bass_kernel_spmd
from concourse.masks import make_identity

B, L, H, D, M = 2, 4096, 8, 64, 256
EPS = 1e-3
C = 128                 # chunk length
NCH = L // C            # 32 chunks
NCORES = 8
PAIRS_PER_CORE = (B * H) // NCORES  # 2
F32 = mybir.dt.float32
BF16 = mybir.dt.bfloat16

# kv PSUM packing: m0 at cols [0:65], m1 at cols [68:133] (16B-aligned)
KV1 = 68
KVW = 136


def _legalize_sync_waits(nc):
    """Split multi-wait instructions into preceding single-wait
    EventSemaphore ops on the same engine (same-engine execution is
    in-order, so sequential waits == AND of waits)."""
    for f in nc.m.functions:
        for b in f.blocks:
            insts = b.instructions
            new = []
            dirty = False
            for ins in insts:
                si = ins.sync_info
                if si is not None and si.on_wait is not None and len(si.on_wait) > 1:
                    waits = list(si.on_wait)
                    for j, wt in enumerate(waits[:-1]):
                        es = mybir.InstEventSemaphore(
                            name=f"{ins.name}_xw{j}",
                            engine=ins.engine,
                            ins=[],
                            outs=[],
                            sync_info=mybir.SyncInfo(on_wait=[wt], on_update=[]),
                        )
                        new.append(es)
                    ins.sync_info = mybir.SyncInfo(
                        on_wait=[waits[-1]], on_update=list(si.on_update or [])
                    )
                    dirty = True
                if si is not None and si.on_update is not None and len(si.on_update) > 1:
                    raise AssertionError(
                        f"multi-update on {ins.name} ({ins.opcode}) unsupported"
                    )
                new.append(ins)
            if dirty:
                b.instructions = new


def _build_program(legalize=True):
    nc = bass.Bass()

    qk_in = []
    outs = []
    for p in range(PAIRS_PER_CORE):
        qd = nc.dram_tensor(f"q{p}", [L, D], BF16, kind="ExternalInput")
        kd = nc.dram_tensor(f"k{p}", [L, D], BF16, kind="ExternalInput")
        vd = nc.dram_tensor(f"v{p}", [L, D + 1], BF16, kind="ExternalInput")
        qk_in.append((qd, kd, vd))
        outs.append(nc.dram_tensor(f"o{p}", [L, D], F32, kind="ExternalOutput"))
    cos2_d = nc.dram_tensor("cos2", [L, 2 * D], BF16, kind="ExternalInput")
    sin2_d = nc.dram_tensor("sin2", [L, 2 * D], BF16, kind="ExternalInput")
    projt_d = nc.dram_tensor("projt", [D, M], BF16, kind="ExternalInput")
    mask_d = nc.dram_tensor("maskat", [C, C], F32, kind="ExternalInput")

    with tile.TileContext(nc) as tc:
        with (
            tc.tile_pool(name="consts", bufs=1) as consts,
            tc.tile_pool(name="stream", bufs=8) as stream,
            tc.tile_pool(name="featA", bufs=7) as featA,     # A->B carried tiles
            tc.tile_pool(name="featL", bufs=4) as featL,     # A-local tiles
            tc.tile_pool(name="outp", bufs=4) as outp,       # B-local tiles
            tc.tile_pool(name="psF", bufs=2, space="PSUM") as psF,
            tc.tile_pool(name="psT", bufs=1, space="PSUM") as psT,
            tc.tile_pool(name="psO", bufs=2, space="PSUM") as psO,
            tc.tile_pool(name="pskv", bufs=1, space="PSUM") as pskv,
        ):
            # ---- constants ----
            cos_sb = consts.tile([128, NCH, 2 * D], BF16)
            sin_sb = consts.tile([128, NCH, 2 * D], BF16)
            nc.sync.dma_start(cos_sb[:], cos2_d.rearrange("(c p) j -> p c j", p=128))
            nc.sync.dma_start(sin_sb[:], sin2_d.rearrange("(c p) j -> p c j", p=128))
            projt2 = consts.tile([128, M], BF16)
            nc.sync.dma_start(projt2[0:D, :], projt_d[:])
            nc.sync.dma_start(projt2[D : 2 * D, :], projt_d[:])
            maskat = consts.tile([C, C], F32)
            nc.sync.dma_start(maskat[:], mask_d[:])
            ident = consts.tile([128, 128], BF16)
            make_identity(nc, ident[:])

            kv_ps = [
                pskv.tile([128, KVW], F32, name=f"kvps{p}", tag=f"kv{p}")
                for p in range(PAIRS_PER_CORE)
            ]

            def stage_a(p, ci):
                """Frontend: load, rotary, transpose, features, relu, AT."""
                qd, kd, vd = qk_in[p]
                lo = ci * C

                xqk = stream.tile([128, 128], BF16, tag="xqk", name=f"xqk{p}_{ci}")
                nc.sync.dma_start(xqk[:, 0:D], qd[lo : lo + C, :])
                nc.sync.dma_start(xqk[:, D : 2 * D], kd[lo : lo + C, :])
                v_aug = featA.tile([128, D + 1], BF16, tag="vaug", name=f"va{p}_{ci}")
                nc.sync.dma_start(v_aug[:], vd[lo : lo + C, :])

                # rotary: rot = x*cos2 + swap(x)*sin2alt
                cslice = cos_sb[:, ci, :]
                sslice = sin_sb[:, ci, :]
                x_sw = xqk.rearrange("p (t two) -> p t two", two=2)[:, :, ::-1]
                t2 = stream.tile([128, 128], BF16, tag="t2", name=f"t2{p}_{ci}")
                nc.gpsimd.tensor_tensor(
                    t2[:].rearrange("p (t two) -> p t two", two=2),
                    x_sw,
                    sslice.rearrange("p (t two) -> p t two", two=2),
                    mybir.AluOpType.mult,
                )
                t1 = stream.tile([128, 128], BF16, tag="t1", name=f"t1{p}_{ci}")
                nc.vector.tensor_tensor(t1[:], xqk[:], cslice, mybir.AluOpType.mult)
                rot = stream.tile([128, 128], BF16, tag="rot", name=f"rot{p}_{ci}")
                nc.gpsimd.tensor_tensor(rot[:], t1[:], t2[:], mybir.AluOpType.add)

                # PE transpose: rotT rows 0:63 = qT, rows 64:127 = kT
                pt = psT.tile([128, 128], BF16, tag="pt", name=f"pt{p}_{ci}")
                nc.tensor.transpose(pt[:], rot[:], ident[:])
                rotT = featL.tile([128, 128], BF16, tag="rotT", name=f"rT{p}_{ci}")
                nc.scalar.copy(rotT[:], pt[:])

                # features: q on PE rows 0:63 -> psum bank "pfq";
                # k + kp on rows 64:127 -> bank "pfk" (concurrent row groups
                # must drain into different banks). AT shares the pfq bank.
                ps_fq = psF.tile([128, 384], F32, tag="pfq", name=f"pfq{p}_{ci}")
                ps_fk = psF.tile([128, 512], F32, tag="pfk", name=f"pfk{p}_{ci}", bufs=1)
                for m in range(2):
                    nc.tensor.matmul(
                        ps_fq[:, m * 128 : (m + 1) * 128],
                        projt2[0:D, m * 128 : (m + 1) * 128],
                        rotT[0:D, :],
                        start=True, stop=True,
                    )
                    nc.tensor.matmul(
                        ps_fk[:, m * 128 : (m + 1) * 128],
                        projt2[D : 2 * D, m * 128 : (m + 1) * 128],
                        rotT[D : 2 * D, :],
                        start=True, stop=True,
                    )
                if ci < NCH - 1:
                    # kp[C, M] (lhsT of the KV update), k row-group
                    nc.tensor.matmul(
                        ps_fk[:, 256:512],
                        rotT[D : 2 * D, :],
                        projt2[D : 2 * D, :],
                        start=True, stop=True,
                    )

                fsb = featA.tile([128, 512], BF16, tag="fsb", name=f"fsb{p}_{ci}")
                nc.vector.tensor_scalar(
                    fsb[:, 0:256], ps_fq[:, 0:256], 0.0, EPS,
                    mybir.AluOpType.max, mybir.AluOpType.add,
                )
                nc.vector.tensor_scalar(
                    fsb[:, 256:512], ps_fk[:, 0:256], 0.0, EPS,
                    mybir.AluOpType.max, mybir.AluOpType.add,
                )
                qpT = [fsb[:, 0:128], fsb[:, 128:256]]
                kpT = [fsb[:, 256:384], fsb[:, 384:512]]
                kp_sb = None
                if ci < NCH - 1:
                    kp_sb = featA.tile([C, M], BF16, tag="kpsb", name=f"kp{p}_{ci}")
                    nc.vector.tensor_scalar(
                        kp_sb[:], ps_fk[:, 256:512], 0.0, EPS,
                        mybir.AluOpType.max, mybir.AluOpType.add,
                    )

                # AT = kp qp^T (this chunk), causal mask
                ps_a = ps_fq[:, 256:384]
                nc.tensor.matmul(ps_a, kpT[0], qpT[0], start=True, stop=False)
                nc.tensor.matmul(ps_a, kpT[1], qpT[1], start=False, stop=True)
                at_sb = featA.tile([C, C], BF16, tag="atsb", name=f"at{p}_{ci}")
                nc.vector.tensor_tensor(
                    at_sb[:], ps_a, maskat[:], mybir.AluOpType.mult
                )
                return qpT, kp_sb, at_sb, v_aug

            def stage_b(p, ci, qpT, kp_sb, at_sb, v_aug):
                """Backend: KV snapshot, num/den, KV update, divide, store."""
                od = outs[p]
                kv = kv_ps[p]
                lo = ci * C

                if ci > 0:
                    kv_sb = outp.tile([128, KVW], BF16, tag="kvsb", name=f"kvs{p}_{ci}")
                    nc.scalar.copy(
                        kv_sb[:, 0 : KV1 + D + 1], kv[:, 0 : KV1 + D + 1]
                    )

                po = psO.tile([C, D + 1], F32, tag="po", name=f"po{p}_{ci}")
                if ci > 0:
                    nc.tensor.matmul(
                        po[:], qpT[0], kv_sb[:, 0 : D + 1], start=True, stop=False
                    )
                    nc.tensor.matmul(
                        po[:], qpT[1], kv_sb[:, KV1 : KV1 + D + 1],
                        start=False, stop=False,
                    )
                    nc.tensor.matmul(
                        po[:], at_sb[:], v_aug[:], start=False, stop=True
                    )
                else:
                    nc.tensor.matmul(
                        po[:], at_sb[:], v_aug[:], start=True, stop=True
                    )

                # KV += kp^T v_aug (PSUM accumulate across chunks)
                if ci < NCH - 1:
                    for m in range(2):
                        nc.tensor.matmul(
                            kv[:, m * KV1 : m * KV1 + D + 1],
                            kp_sb[:, m * 128 : (m + 1) * 128],
                            v_aug[:],
                            start=(ci == 0 and m == 0),
                            stop=True,
                            skip_group_check=True,
                        )

                rec = outp.tile([C, 1], F32, tag="rec", name=f"rec{p}_{ci}")
                nc.vector.reciprocal(rec[:], po[:, D : D + 1])
                osb = outp.tile([C, D], F32, tag="osb", name=f"osb{p}_{ci}")
                nc.scalar.activation(
                    osb[:], po[:, 0:D],
                    mybir.ActivationFunctionType.Copy,
                    bias=0.0, scale=rec[:],
                )
                nc.scalar.dma_start(od[lo : lo + C, :], osb[:])

            # software pipeline: A(ci) one chunk ahead of B(ci-1)
            DEPTH = 1
            pend = {}
            for ci in range(NCH):
                for p in range(PAIRS_PER_CORE):
                    pend[(p, ci)] = stage_a(p, ci)
                if ci >= DEPTH:
                    for p in range(PAIRS_PER_CORE):
                        stage_b(p, ci - DEPTH, *pend.pop((p, ci - DEPTH)))
            for ci in range(NCH - DEPTH, NCH):
                for p in range(PAIRS_PER_CORE):
                    stage_b(p, ci, *pend.pop((p, ci)))

    if legalize:
        _legalize_sync_waits(nc)
    return nc


_PROGRAM_CACHE = {}


def _get_program():
    if "nc" not in _PROGRAM_CACHE:
        _PROGRAM_CACHE["nc"] = _build_program()
    return _PROGRAM_CACHE["nc"]


def _host_prep(sinu_pos, proj):
    bf = ml_dtypes.bfloat16
    sinu = np.asarray(sinu_pos, np.float32)[0]          # [L, D]
    proj = np.asarray(proj, np.float32)                 # [M, D]
    half = D // 2
    sin_i = np.repeat(sinu[:, :half], 2, axis=-1)       # [L, D]
    cos_i = np.repeat(sinu[:, half:], 2, axis=-1)
    sinalt = sin_i.copy()
    sinalt[:, 0::2] *= -1.0
    cos2 = np.ascontiguousarray(np.concatenate([cos_i, cos_i], axis=1)).astype(bf)
    sin2 = np.ascontiguousarray(np.concatenate([sinalt, sinalt], axis=1)).astype(bf)
    projt = np.ascontiguousarray(proj.T / np.sqrt(np.float32(M))).astype(bf)
    maskat = np.triu(np.ones((C, C), np.float32))
    return cos2, sin2, projt, maskat


def build_in_maps(q, k, v, sinu_pos, proj):
    bf = ml_dtypes.bfloat16
    q = np.asarray(q, np.float32)
    k = np.asarray(k, np.float32)
    v = np.asarray(v, np.float32)
    cos2, sin2, projt, maskat = _host_prep(sinu_pos, proj)
    ones_col = np.ones((L, 1), np.float32)
    pairs = [(b, h) for b in range(B) for h in range(H)]
    in_maps = []
    for core in range(NCORES):
        im = {"cos2": cos2, "sin2": sin2, "projt": projt, "maskat": maskat}
        for p in range(PAIRS_PER_CORE):
            b, h = pairs[core * PAIRS_PER_CORE + p]
            im[f"q{p}"] = np.ascontiguousarray(q[b, :, h, :]).astype(bf)
            im[f"k{p}"] = np.ascontiguousarray(k[b, :, h, :]).astype(bf)
            im[f"v{p}"] = np.ascontiguousarray(
                np.concatenate([v[b, :, h, :], ones_col], axis=1)
            ).astype(bf)
        in_maps.append(im)
    return in_maps


def kernel(q, k, v, sinu_pos, proj):
    nc = _get_program()
    in_maps = build_in_maps(q, k, v, sinu_pos, proj)
    res = run_bass_kernel_spmd(nc, in_maps, core_ids=list(range(NCORES)))

    pairs = [(b, h) for b in range(B) for h in range(H)]
    out = np.empty((B, L, H, D), np.float32)
    for core in range(NCORES):
        for p in range(PAIRS_PER_CORE):
            b, h = pairs[core * PAIRS_PER_CORE + p]
            out[b, :, h, :] = res.results[core][f"o{p}"]
    return out
